# revision 15
# baseline (speedup 1.0000x reference)
"""Trainium2 Bass kernel for unscaled dot-product attention.

Shapes (hardcoded): query/key/value [2048, 2, 16, 64] fp32.
  scores = einsum('sbnh,tbnh->bnst', q, k)   (UNscaled)
  probs  = softmax(scores, axis=-1)
  out    = einsum('bnst,tbnh->sbnh', probs, v).reshape(2048, 2, 1024)

Sharding: the 32 (b, n) head-slices are split 4-per-core across 8 cores
(core c -> b = c//4, heads 4*(c%4) .. +4). Each core computes attention
for its 4 heads independently; no cross-device communication.

Device-side strategy (per core, heads processed in 2 pairs):
  - The host packs, per head pair, ONE contiguous SBUF-shaped slab
    [128, 6208] = [Q^T pair | K^T pair | V' blocks] so each pair needs a
    single fully-contiguous DMA (one completion semaphore -> the fp32r
    self-loading matmuls never need more than one sync wait, which is
    all the LDWEIGHTS slot supports).
  - Q^T/K^T are [head*64+h, s]; V' is [t, 66] per t-block per head with
    a ones column (so the PV matmul produces the softmax denominator
    for free) and a zero pad column (fp32r weights need an even count).
  - All matmul operands are float32r (fp32 layout, 11-bit mantissa,
    1 PE cycle/row vs 4 for plain fp32). The host pre-rounds inputs to
    the fp32r grid so DMA-ed bits are already "rounded to FP32r".
  - scores are computed TRANSPOSED: scoresT[t_block, s] = K^T.T @ Q^T
    per 128-t block, two heads packed into the 128 contraction rows of
    the PE array (head A partitions 0-63, head B 64-127, concurrent
    matmuls via row tile_position).
  - exp() runs on the Scalar (ACT) engine straight out of PSUM, with no
    max-subtraction: unscaled fp32 scores max out around +-66, so exp
    stays comfortably inside fp32 range (mathematically identical to
    softmax with max-subtraction).
  - PV accumulates CT[66, s] = V'.T @ expT over the 16 t-blocks in PSUM
    (rows 0-63 context^T, row 64 = denominator, row 65 = padding).
  - Normalization: recip of the denominator row, broadcast across
    partitions with a tiny K=1 fp32 matmul, one DVE multiply, and the
    [64, s] result DMAs straight to the per-head transposed output,
    which the host transposes back.
"""

import numpy as np

SQ, B, NHEADS, HN = 2048, 2, 16, 64
N_CORES = 8
HEADS_PER_CORE = 4
VW = 66                     # V' columns per head (64 V + ones + pad)


def _slab_geom(qk_dt, pv_dt):
    """Slab section widths in 4-byte units."""
    qk_w = SQ if qk_dt == "float32r" else SQ // 2
    v_w = 16 * 2 * VW if pv_dt == "float32r" else 16 * VW
    return qk_w, v_w, 2 * qk_w + v_w

_CACHE = {}


def _round_fp32r(x):
    """Round fp32 array to the fp32r grid (11 explicit mantissa bits,
    round-to-nearest-even, low 12 bits zero)."""
    u = np.ascontiguousarray(x, np.float32).view(np.uint32)
    lsb = (u >> 12) & 1
    u = (u + 0x7FF + lsb) & 0xFFFFF000
    return u.astype(np.uint32).view(np.float32)


def _build_program(reps=1, qk_dt="float32r", pv_dt="float32r"):
    from contextlib import ExitStack

    import concourse.bacc as bacc
    import concourse.mybir as mybir
    import concourse.tile as tile

    f32 = mybir.dt.float32
    f32r = mybir.dt.float32r
    qkd = getattr(mybir.dt, qk_dt)
    pvd = getattr(mybir.dt, pv_dt)
    EXP = mybir.ActivationFunctionType.Exp

    nc = bacc.Bacc("TRN2", target_bir_lowering=False, debug=False,
                   num_devices=N_CORES)

    qk_w, v_w, slab_w = _slab_geom(qk_dt, pv_dt)
    ins = nc.dram_tensor("ins", [2, 128, slab_w], f32r,
                         kind="ExternalInput").ap()
    outT = nc.dram_tensor("outT", [256, SQ], f32, kind="ExternalOutput").ap()

    SCH = 512            # s-chunk processed per inner loop
    NCH = SQ // SCH      # 4 chunks
    NT = SQ // 128       # 16 t-blocks

    with tile.TileContext(nc) as tc, ExitStack() as ctx:
        const_pool = ctx.enter_context(tc.tile_pool(name="const", bufs=1))
        slab_pool = ctx.enter_context(tc.tile_pool(name="slab", bufs=2))
        ex_pool = ctx.enter_context(tc.tile_pool(name="ex", bufs=3))
        fin_pool = ctx.enter_context(tc.tile_pool(name="fin", bufs=2))
        ot_pool = ctx.enter_context(tc.tile_pool(name="ot", bufs=2))
        # PSUM budget (8 banks of [128 x 512 fp32]):
        #   scores 2 bufs x 2 banks = 4, CT 2, bcast 2  -> 8
        ps_sc = ctx.enter_context(tc.tile_pool(name="ps_sc", bufs=2, space="PSUM"))
        ps_ct = ctx.enter_context(tc.tile_pool(name="ps_ct", bufs=1, space="PSUM"))
        ps_bc = ctx.enter_context(tc.tile_pool(name="ps_bc", bufs=1, space="PSUM"))

        ones_col = const_pool.tile([1, 128], f32)
        nc.vector.memset(ones_col[:], 1.0)

        rep_cm = tc.For_i(0, reps, 1) if reps != 1 else None
        if rep_cm is not None:
            ctx.enter_context(rep_cm)

        # Load both pair slabs up front (bufs=2): pair 1 prefetches during
        # pair 0 compute.
        pair = []
        for g in range(2):
            slab = slab_pool.tile([128, slab_w], f32r, tag="slab")
            nc.sync.dma_start(out=slab[:], in_=ins[g])
            QT2 = slab[:, 0:qk_w]
            KT2 = slab[:, qk_w:2 * qk_w]
            vsec = slab[:, 2 * qk_w:]
            if qk_dt != "float32r":
                QT2 = QT2.bitcast(qkd)
                KT2 = KT2.bitcast(qkd)
            if pv_dt != "float32r":
                vsec = vsec.bitcast(pvd)
            v3 = vsec.rearrange("p (j c) -> p j c", c=2 * VW)
            pair.append((QT2, KT2, v3))

        # Software-pipelined emission over flat steps (g, c, j): the QK pair
        # for step s+1 is emitted BEFORE PV of step s, so the in-order PE
        # queue never stalls behind exp(s) while QK(s+1) is ready to run.
        steps = [(g, c, j) for g in range(2) for c in range(NCH)
                 for j in range(NT)]

        def emit_qk(s):
            g, c, j = steps[s]
            QT2, KT2, _ = pair[g]
            s0 = c * SCH
            sc = ps_sc.tile([128, 1024], f32, tag="sc")
            nc.tensor.matmul(
                sc[:, 0:512],
                lhsT=KT2[0:64, j * 128:(j + 1) * 128],
                rhs=QT2[0:64, s0:s0 + SCH],
                start=True, stop=True)
            nc.tensor.matmul(
                sc[:, 512:1024],
                lhsT=KT2[64:128, j * 128:(j + 1) * 128],
                rhs=QT2[64:128, s0:s0 + SCH],
                start=True, stop=True)
            return sc

        # deferred finalize state: (g, c, CTs, rec) awaiting bc + muls + DMA
        pend = []

        def emit_finalize2():
            g, c, CTs, rec = pend.pop(0)
            s0 = c * SCH
            bc = ps_bc.tile([128, 1024], f32, tag="bc")
            nc.tensor.matmul(bc[:, 0:512], lhsT=ones_col,
                             rhs=rec[:, 0:512], start=True, stop=True)
            nc.tensor.matmul(bc[:, 512:1024], lhsT=ones_col,
                             rhs=rec[:, 512:1024], start=True, stop=True)
            OTa = ot_pool.tile([64, 512], f32, tag="ota")
            nc.vector.tensor_mul(OTa[:], CTs[0:64, 0:512], bc[0:64, 0:512])
            OTb = ot_pool.tile([64, 512], f32, tag="otb")
            nc.vector.tensor_mul(OTb[:], CTs[0:64, 512:1024],
                                 bc[0:64, 512:1024])
            nc.sync.dma_start(
                out=outT[(2 * g) * 64:(2 * g + 1) * 64, s0:s0 + SCH],
                in_=OTa[:])
            nc.sync.dma_start(
                out=outT[(2 * g + 1) * 64:(2 * g + 2) * 64, s0:s0 + SCH],
                in_=OTb[:])

        sc_cur = emit_qk(0)
        CT = None
        for s, (g, c, j) in enumerate(steps):
            _, _, v3 = pair[g]
            if j == 0:
                CT = ps_ct.tile([128, 1024], f32, tag="ct")
            sc_next = emit_qk(s + 1) if s + 1 < len(steps) else None
            ex = ex_pool.tile([128, 1024], pvd, tag="ex")
            nc.scalar.activation(ex[:], sc_cur[:], EXP)
            nc.tensor.matmul(
                CT[0:VW, 0:512],
                lhsT=v3[:, j, 0:VW],
                rhs=ex[:, 0:512],
                start=(j == 0), stop=(j == NT - 1))
            nc.tensor.matmul(
                CT[0:VW, 512:1024],
                lhsT=v3[:, j, VW:2 * VW],
                rhs=ex[:, 512:1024],
                start=(j == 0), stop=(j == NT - 1))
            sc_cur = sc_next
            # emit the deferred (PE-side) finalize of the previous chunk
            # mid-chunk, when its DVE inputs are long since ready
            if j == 8 and pend:
                emit_finalize2()
            if j == NT - 1:
                # finalize part 1 (DVE only): evacuate CT, reciprocal
                CTs = fin_pool.tile([65, 1024], f32, tag="cts")
                nc.vector.tensor_copy(CTs[:], CT[0:65, :])
                rec = fin_pool.tile([1, 1024], f32, tag="rec")
                nc.vector.reciprocal(rec[:], CTs[64:65, :])
                pend.append((g, c, CTs, rec))
        while pend:
            emit_finalize2()
    nc.compile()
    return nc


def get_nc(reps=1, qk_dt="float32r", pv_dt="float32r"):
    key = ("nc", reps, qk_dt, pv_dt)
    if key not in _CACHE:
        _CACHE[key] = _build_program(reps, qk_dt, pv_dt)
    return _CACHE[key]


def _pack_section(arr, dt_name):
    """arr: fp32 [..., n]. Returns a float32-typed view of the section data
    in its on-device dtype (f32r: same width; bf16: half width)."""
    if dt_name == "float32r":
        return _round_fp32r(arr)
    import ml_dtypes
    bf = np.ascontiguousarray(arr, np.float32).astype(ml_dtypes.bfloat16)
    out = np.zeros(arr.shape[:-1] + (arr.shape[-1] // 2,), np.float32)
    out.view(np.uint16)[...] = bf.view(np.uint16)
    return out


def make_in_maps(query, key, value, qk_dt="float32r", pv_dt="float32r"):
    """Host-side sharding + layout prep. Returns list of per-core input maps."""
    query = np.asarray(query, dtype=np.float32)
    key = np.asarray(key, dtype=np.float32)
    value = np.asarray(value, dtype=np.float32)
    in_maps = []
    for c in range(N_CORES):
        b = c // 4
        n0 = HEADS_PER_CORE * (c % 4)
        q = query[:, b, n0:n0 + 4, :]   # [2048, 4, 64]
        k = key[:, b, n0:n0 + 4, :]
        v = value[:, b, n0:n0 + 4, :]
        # [4, 64, 2048] -> per pair rows
        qt = _pack_section(q.transpose(1, 2, 0).reshape(2, 128, SQ), qk_dt)
        kt = _pack_section(k.transpose(1, 2, 0).reshape(2, 128, SQ), qk_dt)
        # V' [2048, 4, 66] -> [16, 128, 2 pairs, 132] -> [2, 128, 16*132]
        vp = np.concatenate(
            [v, np.ones((SQ, 4, 1), np.float32),
             np.zeros((SQ, 4, 1), np.float32)], axis=2)
        vp = vp.reshape(16, 128, 2, 2 * VW).transpose(2, 1, 0, 3)
        vp = _pack_section(vp.reshape(2, 128, 16 * 2 * VW), pv_dt)
        slab = np.concatenate([qt, kt, vp], axis=2)
        in_maps.append({"ins": np.ascontiguousarray(slab)})
    return in_maps


def assemble_output(results):
    """results: list of per-core {name: array} dicts -> full [2048, 2, 1024]."""
    out = np.empty((SQ, B, NHEADS, HN), np.float32)
    for c in range(N_CORES):
        b = c // 4
        n0 = HEADS_PER_CORE * (c % 4)
        oT = np.asarray(results[c]["outT"])  # [256, 2048]
        out[:, b, n0:n0 + 4, :] = oT.reshape(4, HN, SQ).transpose(2, 0, 1)
    return out.reshape(SQ, B, NHEADS * HN)


def kernel(query, key, value):
    try:
        from concourse.bass_utils import run_bass_kernel_spmd
    except ImportError:
        import sys
        sys.path.insert(0, "/opt/trn_rl_repo")
        from concourse.bass_utils import run_bass_kernel_spmd

    nc = get_nc()
    in_maps = make_in_maps(query, key, value)
    res = run_bass_kernel_spmd(nc, in_maps, list(range(N_CORES)))
    return assemble_output(res.results)


# revision 27
# speedup vs baseline: 1.0331x; 1.0331x over previous
"""Trainium2 Bass kernel for unscaled dot-product attention.

Shapes (hardcoded): query/key/value [2048, 2, 16, 64] fp32.
  scores = einsum('sbnh,tbnh->bnst', q, k)   (UNscaled)
  probs  = softmax(scores, axis=-1)
  out    = einsum('bnst,tbnh->sbnh', probs, v).reshape(2048, 2, 1024)

Sharding: the 32 (b, n) head-slices are split 4-per-core across 8 cores
(core c -> b = c//4, heads 4*(c%4) .. +4). Each core computes attention
for its 4 heads independently; no cross-device communication.

Device-side strategy (per core, heads processed in 2 pairs):
  - The host packs, per head pair, ONE contiguous SBUF-shaped slab
    [128, 6208] = [Q^T pair | K^T pair | V' blocks] so each pair needs a
    single fully-contiguous DMA (one completion semaphore -> the fp32r
    self-loading matmuls never need more than one sync wait, which is
    all the LDWEIGHTS slot supports).
  - Q^T/K^T are [head*64+h, s]; V' is [t, 66] per t-block per head with
    a ones column (so the PV matmul produces the softmax denominator
    for free) and a zero pad column (fp32r weights need an even count).
  - All matmul operands are float32r (fp32 layout, 11-bit mantissa,
    1 PE cycle/row vs 4 for plain fp32). The host pre-rounds inputs to
    the fp32r grid so DMA-ed bits are already "rounded to FP32r".
  - scores are computed TRANSPOSED: scoresT[t_block, s] = K^T.T @ Q^T
    per 128-t block, two heads packed into the 128 contraction rows of
    the PE array (head A partitions 0-63, head B 64-127, concurrent
    matmuls via row tile_position).
  - exp() runs on the Scalar (ACT) engine straight out of PSUM, with no
    max-subtraction: unscaled fp32 scores max out around +-66, so exp
    stays comfortably inside fp32 range (mathematically identical to
    softmax with max-subtraction).
  - PV accumulates CT[66, s] = V'.T @ expT over the 16 t-blocks in PSUM
    (rows 0-63 context^T, row 64 = denominator, row 65 = padding).
  - Normalization: recip of the denominator row, broadcast across
    partitions with a tiny K=1 fp32 matmul, one DVE multiply, and the
    [64, s] result DMAs straight to the per-head transposed output,
    which the host transposes back.
"""

import numpy as np

SQ, B, NHEADS, HN = 2048, 2, 16, 64
N_CORES = 8
HEADS_PER_CORE = 4
VW = 66                     # V' columns per head (64 V + ones + pad)


def _slab_geom(qk_dt, pv_dt):
    """Slab section widths in 4-byte units."""
    qk_w = SQ if qk_dt == "float32r" else SQ // 2
    v_w = 16 * 2 * VW if pv_dt == "float32r" else 16 * VW
    return qk_w, v_w, 2 * qk_w + v_w

_CACHE = {}


def _round_fp32r(x):
    """Round fp32 array to the fp32r grid (11 explicit mantissa bits,
    round-to-nearest-even, low 12 bits zero)."""
    u = np.ascontiguousarray(x, np.float32).view(np.uint32)
    lsb = (u >> 12) & 1
    u = (u + 0x7FF + lsb) & 0xFFFFF000
    return u.astype(np.uint32).view(np.float32)


def _build_program(reps=1, qk_dt="float32r", pv_dt="float32r",
                   fin_mode="full", dma_hoist=False):
    from contextlib import ExitStack

    import concourse.bacc as bacc
    import concourse.mybir as mybir
    import concourse.tile as tile

    f32 = mybir.dt.float32
    f32r = mybir.dt.float32r
    qkd = getattr(mybir.dt, qk_dt)
    pvd = getattr(mybir.dt, pv_dt)
    EXP = mybir.ActivationFunctionType.Exp

    nc = bacc.Bacc("TRN2", target_bir_lowering=False, debug=False,
                   num_devices=N_CORES)

    qk_w, v_w, slab_w = _slab_geom(qk_dt, pv_dt)
    ins = nc.dram_tensor("ins", [2, 128, slab_w], f32r,
                         kind="ExternalInput").ap()
    outT = nc.dram_tensor("outT", [256, SQ], f32, kind="ExternalOutput").ap()

    SCH = 512            # s-chunk processed per inner loop
    NCH = SQ // SCH      # 4 chunks
    NT = SQ // 128       # 16 t-blocks

    with tile.TileContext(nc) as tc, ExitStack() as ctx:
        const_pool = ctx.enter_context(tc.tile_pool(name="const", bufs=1))
        slab_pool = ctx.enter_context(tc.tile_pool(name="slab", bufs=2))
        ex_pool = ctx.enter_context(tc.tile_pool(name="ex", bufs=3))
        fin_pool = ctx.enter_context(tc.tile_pool(name="fin", bufs=2))
        ot_pool = ctx.enter_context(tc.tile_pool(name="ot", bufs=2))
        # PSUM budget (8 banks of [128 x 512 fp32]):
        #   score ring 6 banks (X: 4 = two steps, Y: 2 = one step), CT 2 -> 8
        ps_sc = ctx.enter_context(tc.tile_pool(name="ps_sc", bufs=1, space="PSUM"))
        ps_ct = ctx.enter_context(tc.tile_pool(name="ps_ct", bufs=1, space="PSUM"))

        rep_cm = tc.For_i(0, reps, 1) if reps != 1 else None

        def load_pairs():
            # Load both pair slabs up front (bufs=2): pair 1 prefetches
            # during pair 0 compute.
            pair = []
            for g in range(2):
                slab = slab_pool.tile([128, slab_w], f32r, tag="slab")
                nc.sync.dma_start(out=slab[:], in_=ins[g])
                QT2 = slab[:, 0:qk_w]
                KT2 = slab[:, qk_w:2 * qk_w]
                vsec = slab[:, 2 * qk_w:]
                if qk_dt != "float32r":
                    QT2 = QT2.bitcast(qkd)
                    KT2 = KT2.bitcast(qkd)
                if pv_dt != "float32r":
                    vsec = vsec.bitcast(pvd)
                v3 = vsec.rearrange("p (j c) -> p j c", c=2 * VW)
                pair.append((QT2, KT2, v3))
            return pair

        if dma_hoist:
            pair = load_pairs()
        if rep_cm is not None:
            ctx.enter_context(rep_cm)
        if not dma_hoist:
            pair = load_pairs()

        # Software-pipelined emission over flat steps (g, c, j). Scores go
        # into a manual 6-bank PSUM ring: region X holds TWO steps (4 banks,
        # one [128, 2048] exp), region Y holds ONE (2 banks, [128, 1024]
        # exp); X/Y alternate over the whole stream, cutting ACT
        # per-instruction overhead by ~1/3. The QK fills for super-step
        # k+1 are emitted BEFORE the PVs of super-step k so the in-order PE
        # queue never idles behind exp(k).
        steps = [(g, c, j) for g in range(2) for c in range(NCH)
                 for j in range(NT)]
        supers = []
        i, use_x = 0, True
        while i < len(steps):
            n = 2 if use_x else 1
            supers.append((use_x, list(range(i, min(i + n, len(steps))))))
            i += n
            use_x = not use_x

        ring = ps_sc.tile([128, 3072], f32, tag="ring")

        def emit_qk_super(k):
            use_x, ss = supers[k]
            base = 0 if use_x else 2048
            for sl, s in enumerate(ss):
                g, c, j = steps[s]
                QT2, KT2, _ = pair[g]
                s0 = c * SCH
                off = base + sl * 1024
                nc.tensor.matmul(
                    ring[:, off:off + 512],
                    lhsT=KT2[0:64, j * 128:(j + 1) * 128],
                    rhs=QT2[0:64, s0:s0 + SCH],
                    start=True, stop=True)
                nc.tensor.matmul(
                    ring[:, off + 512:off + 1024],
                    lhsT=KT2[64:128, j * 128:(j + 1) * 128],
                    rhs=QT2[64:128, s0:s0 + SCH],
                    start=True, stop=True)

        # deferred finalize state: (g, c, CTs, rec) awaiting bc + muls + DMA
        pend = []

        def emit_finalize2():
            g, c, CTs, rec = pend.pop(0)
            s0 = c * SCH
            # broadcast 1/denom across partitions on the (idle) GPSIMD
            bc = fin_pool.tile([128, 1024], f32, tag="bc")
            nc.gpsimd.partition_broadcast(bc[:], rec[:])
            OTa = ot_pool.tile([64, 512], f32, tag="ota")
            nc.vector.tensor_mul(OTa[:], CTs[0:64, 0:512], bc[0:64, 0:512])
            OTb = ot_pool.tile([64, 512], f32, tag="otb")
            nc.vector.tensor_mul(OTb[:], CTs[0:64, 512:1024],
                                 bc[0:64, 512:1024])
            nc.sync.dma_start(
                out=outT[(2 * g) * 64:(2 * g + 1) * 64, s0:s0 + SCH],
                in_=OTa[:])
            nc.sync.dma_start(
                out=outT[(2 * g + 1) * 64:(2 * g + 2) * 64, s0:s0 + SCH],
                in_=OTb[:])

        emit_qk_super(0)
        CT = [None]

        def emit_pv(s, ex, exoff):
            g, c, j = steps[s]
            _, _, v3 = pair[g]
            if j == 0:
                ct_t = ps_ct.tile([128, 1024], f32, tag="ct", name="ct_t")
                CT[0] = ct_t
            nc.tensor.matmul(
                CT[0][0:VW, 0:512],
                lhsT=v3[:, j, 0:VW],
                rhs=ex[:, exoff:exoff + 512],
                start=(j == 0), stop=(j == NT - 1))
            nc.tensor.matmul(
                CT[0][0:VW, 512:1024],
                lhsT=v3[:, j, VW:2 * VW],
                rhs=ex[:, exoff + 512:exoff + 1024],
                start=(j == 0), stop=(j == NT - 1))

        def emit_tail(s):
            # per-step chunk bookkeeping: deferred finalize + chunk-end
            # evacuation of CT (DVE-only part)
            g, c, j = steps[s]
            if j == 8 and pend:
                emit_finalize2()
            if j != NT - 1:
                return
            if fin_mode == "none":
                dcons = fin_pool.tile([1, 8], f32, tag="dcons")
                nc.vector.tensor_copy(dcons[:], CT[0][0:1, 0:8])
                nc.sync.dma_start(
                    out=outT[200:201,
                             (g * NCH + c) * 8:(g * NCH + c + 1) * 8],
                    in_=dcons[:])
                return
            CTs = fin_pool.tile([65, 1024], f32, tag="cts")
            nc.vector.tensor_copy(CTs[:], CT[0][0:65, :])
            rec = fin_pool.tile([1, 1024], f32, tag="rec")
            nc.vector.reciprocal(rec[:], CTs[64:65, :])
            if fin_mode == "part1":
                dcons = fin_pool.tile([1, 8], f32, tag="dcons")
                nc.vector.tensor_copy(dcons[:], rec[0:1, 0:8])
                nc.sync.dma_start(
                    out=outT[200:201,
                             (g * NCH + c) * 8:(g * NCH + c + 1) * 8],
                    in_=dcons[:])
                return
            pend.append((g, c, CTs, rec))

        for k, (use_x, ss) in enumerate(supers):
            if k + 1 < len(supers):
                emit_qk_super(k + 1)
            base = 0 if use_x else 2048
            width = 1024 * len(ss)
            ex = ex_pool.tile([128, width], pvd,
                              tag="exX" if use_x else "exY")
            nc.scalar.activation(ex[:], ring[:, base:base + width], EXP)
            for sl, s in enumerate(ss):
                emit_pv(s, ex, sl * 1024)
                emit_tail(s)

        while pend:
            emit_finalize2()
    nc.compile()
    return nc


def get_nc(reps=1, qk_dt="float32r", pv_dt="float32r", fin_mode="full",
           dma_hoist=False):
    key = ("nc", reps, qk_dt, pv_dt, fin_mode, dma_hoist)
    if key not in _CACHE:
        _CACHE[key] = _build_program(reps, qk_dt, pv_dt, fin_mode, dma_hoist)
    return _CACHE[key]


def _pack_section(arr, dt_name):
    """arr: fp32 [..., n]. Returns a float32-typed view of the section data
    in its on-device dtype (f32r: same width; bf16: half width)."""
    if dt_name == "float32r":
        return _round_fp32r(arr)
    import ml_dtypes
    bf = np.ascontiguousarray(arr, np.float32).astype(ml_dtypes.bfloat16)
    out = np.zeros(arr.shape[:-1] + (arr.shape[-1] // 2,), np.float32)
    out.view(np.uint16)[...] = bf.view(np.uint16)
    return out


def make_in_maps(query, key, value, qk_dt="float32r", pv_dt="float32r"):
    """Host-side sharding + layout prep. Returns list of per-core input maps."""
    query = np.asarray(query, dtype=np.float32)
    key = np.asarray(key, dtype=np.float32)
    value = np.asarray(value, dtype=np.float32)
    in_maps = []
    for c in range(N_CORES):
        b = c // 4
        n0 = HEADS_PER_CORE * (c % 4)
        q = query[:, b, n0:n0 + 4, :]   # [2048, 4, 64]
        k = key[:, b, n0:n0 + 4, :]
        v = value[:, b, n0:n0 + 4, :]
        # [4, 64, 2048] -> per pair rows
        qt = _pack_section(q.transpose(1, 2, 0).reshape(2, 128, SQ), qk_dt)
        kt = _pack_section(k.transpose(1, 2, 0).reshape(2, 128, SQ), qk_dt)
        # V' [2048, 4, 66] -> [16, 128, 2 pairs, 132] -> [2, 128, 16*132]
        vp = np.concatenate(
            [v, np.ones((SQ, 4, 1), np.float32),
             np.zeros((SQ, 4, 1), np.float32)], axis=2)
        vp = vp.reshape(16, 128, 2, 2 * VW).transpose(2, 1, 0, 3)
        vp = _pack_section(vp.reshape(2, 128, 16 * 2 * VW), pv_dt)
        slab = np.concatenate([qt, kt, vp], axis=2)
        in_maps.append({"ins": np.ascontiguousarray(slab)})
    return in_maps


def assemble_output(results):
    """results: list of per-core {name: array} dicts -> full [2048, 2, 1024]."""
    out = np.empty((SQ, B, NHEADS, HN), np.float32)
    for c in range(N_CORES):
        b = c // 4
        n0 = HEADS_PER_CORE * (c % 4)
        oT = np.asarray(results[c]["outT"])  # [256, 2048]
        out[:, b, n0:n0 + 4, :] = oT.reshape(4, HN, SQ).transpose(2, 0, 1)
    return out.reshape(SQ, B, NHEADS * HN)


def kernel(query, key, value):
    try:
        from concourse.bass_utils import run_bass_kernel_spmd
    except ImportError:
        import sys
        sys.path.insert(0, "/opt/trn_rl_repo")
        from concourse.bass_utils import run_bass_kernel_spmd

    nc = get_nc()
    in_maps = make_in_maps(query, key, value)
    res = run_bass_kernel_spmd(nc, in_maps, list(range(N_CORES)))
    return assemble_output(res.results)


# revision 31
# speedup vs baseline: 1.5722x; 1.5218x over previous
"""Trainium2 Bass kernel for unscaled dot-product attention.

Shapes (hardcoded): query/key/value [2048, 2, 16, 64] fp32.
  scores = einsum('sbnh,tbnh->bnst', q, k)   (UNscaled)
  probs  = softmax(scores, axis=-1)
  out    = einsum('bnst,tbnh->sbnh', probs, v).reshape(2048, 2, 1024)

Sharding: the 32 (b, n) head-slices are split 4-per-core across 8 cores
(core c -> b = c//4, heads 4*(c%4) .. +4). Each core computes attention
for its 4 heads independently; no cross-device communication.

Device-side strategy (per core, heads processed in 2 pairs):
  - The host packs, per head pair, ONE contiguous SBUF-shaped slab
    [128, 6208] = [Q^T pair | K^T pair | V' blocks] so each pair needs a
    single fully-contiguous DMA (one completion semaphore -> the fp32r
    self-loading matmuls never need more than one sync wait, which is
    all the LDWEIGHTS slot supports).
  - Q^T/K^T are [head*64+h, s]; V' is [t, 66] per t-block per head with
    a ones column (so the PV matmul produces the softmax denominator
    for free) and a zero pad column (fp32r weights need an even count).
  - All matmul operands are float32r (fp32 layout, 11-bit mantissa,
    1 PE cycle/row vs 4 for plain fp32). The host pre-rounds inputs to
    the fp32r grid so DMA-ed bits are already "rounded to FP32r".
  - scores are computed TRANSPOSED: scoresT[t_block, s] = K^T.T @ Q^T
    per 128-t block, two heads packed into the 128 contraction rows of
    the PE array (head A partitions 0-63, head B 64-127, concurrent
    matmuls via row tile_position).
  - exp() runs on the Scalar (ACT) engine straight out of PSUM, with no
    max-subtraction: unscaled fp32 scores max out around +-66, so exp
    stays comfortably inside fp32 range (mathematically identical to
    softmax with max-subtraction).
  - PV accumulates CT[66, s] = V'.T @ expT over the 16 t-blocks in PSUM
    (rows 0-63 context^T, row 64 = denominator, row 65 = padding).
  - Normalization: recip of the denominator row, broadcast across
    partitions with a tiny K=1 fp32 matmul, one DVE multiply, and the
    [64, s] result DMAs straight to the per-head transposed output,
    which the host transposes back.
"""

import numpy as np

SQ, B, NHEADS, HN = 2048, 2, 16, 64
N_CORES = 8
HEADS_PER_CORE = 4
VW = 66                     # V' columns per head (64 V + ones + pad)


def _slab_geom(qk_dt, pv_dt):
    """Slab section widths in 4-byte units."""
    qk_w = SQ if qk_dt == "float32r" else SQ // 2
    v_w = 16 * 2 * VW if pv_dt == "float32r" else 16 * VW
    return qk_w, v_w, 2 * qk_w + v_w

_CACHE = {}


def _round_fp32r(x):
    """Round fp32 array to the fp32r grid (11 explicit mantissa bits,
    round-to-nearest-even, low 12 bits zero)."""
    u = np.ascontiguousarray(x, np.float32).view(np.uint32)
    lsb = (u >> 12) & 1
    u = (u + 0x7FF + lsb) & 0xFFFFF000
    return u.astype(np.uint32).view(np.float32)


def _build_program(reps=1, qk_dt="float32r", pv_dt="float32r",
                   fin_mode="full", dma_hoist=False):
    from contextlib import ExitStack

    import concourse.bacc as bacc
    import concourse.mybir as mybir
    import concourse.tile as tile

    f32 = mybir.dt.float32
    f32r = mybir.dt.float32r
    qkd = getattr(mybir.dt, qk_dt)
    pvd = getattr(mybir.dt, pv_dt)
    EXP = mybir.ActivationFunctionType.Exp

    nc = bacc.Bacc("TRN2", target_bir_lowering=False, debug=False,
                   num_devices=N_CORES)

    qk_w, v_w, slab_w = _slab_geom(qk_dt, pv_dt)
    ins = nc.dram_tensor("ins", [2, 128, slab_w], f32r,
                         kind="ExternalInput").ap()
    outT = nc.dram_tensor("outT", [256, SQ], f32, kind="ExternalOutput").ap()

    SCH = 512            # s-chunk processed per inner loop
    NCH = SQ // SCH      # 4 chunks
    NT = SQ // 128       # 16 t-blocks

    with tile.TileContext(nc) as tc, ExitStack() as ctx:
        const_pool = ctx.enter_context(tc.tile_pool(name="const", bufs=1))
        slab_pool = ctx.enter_context(tc.tile_pool(name="slab", bufs=2))
        ex_pool = ctx.enter_context(tc.tile_pool(name="ex", bufs=3))
        fin_pool = ctx.enter_context(tc.tile_pool(name="fin", bufs=2))
        ot_pool = ctx.enter_context(tc.tile_pool(name="ot", bufs=2))
        # PSUM budget (8 banks of [128 x 512 fp32]):
        #   scores 3 bufs x 2 banks = 6, CT 2  -> 8
        ps_sc = ctx.enter_context(tc.tile_pool(name="ps_sc", bufs=3, space="PSUM"))
        ps_ct = ctx.enter_context(tc.tile_pool(name="ps_ct", bufs=1, space="PSUM"))

        rep_cm = tc.For_i(0, reps, 1) if reps != 1 else None

        def load_pairs():
            # Load both pair slabs up front (bufs=2): pair 1 prefetches
            # during pair 0 compute.
            pair = []
            for g in range(2):
                slab = slab_pool.tile([128, slab_w], f32r, tag="slab")
                nc.sync.dma_start(out=slab[:], in_=ins[g])
                QT2 = slab[:, 0:qk_w]
                KT2 = slab[:, qk_w:2 * qk_w]
                vsec = slab[:, 2 * qk_w:]
                if qk_dt != "float32r":
                    QT2 = QT2.bitcast(qkd)
                    KT2 = KT2.bitcast(qkd)
                if pv_dt != "float32r":
                    vsec = vsec.bitcast(pvd)
                v3 = vsec.rearrange("p (j c) -> p j c", c=2 * VW)
                pair.append((QT2, KT2, v3))
            return pair

        if dma_hoist:
            pair = load_pairs()
        if rep_cm is not None:
            ctx.enter_context(rep_cm)
        if not dma_hoist:
            pair = load_pairs()

        # Software-pipelined emission over flat steps (g, c, j): the QK pair
        # for step s+1 is emitted BEFORE PV of step s, so the in-order PE
        # queue never stalls behind exp(s) while QK(s+1) is ready to run.
        steps = [(g, c, j) for g in range(2) for c in range(NCH)
                 for j in range(NT)]

        def emit_qk(s):
            g, c, j = steps[s]
            QT2, KT2, _ = pair[g]
            s0 = c * SCH
            sc = ps_sc.tile([128, 1024], f32, tag="sc")
            nc.tensor.matmul(
                sc[:, 0:512],
                lhsT=KT2[0:64, j * 128:(j + 1) * 128],
                rhs=QT2[0:64, s0:s0 + SCH],
                start=True, stop=True)
            nc.tensor.matmul(
                sc[:, 512:1024],
                lhsT=KT2[64:128, j * 128:(j + 1) * 128],
                rhs=QT2[64:128, s0:s0 + SCH],
                start=True, stop=True)
            return sc

        # deferred finalize state: (g, c, CTs, rec) awaiting bc + muls + DMA
        pend = []

        def emit_finalize2():
            g, c, CTs, rec = pend.pop(0)
            s0 = c * SCH
            # broadcast 1/denom across partitions on the (idle) GPSIMD
            bc = fin_pool.tile([128, 1024], f32, tag="bc")
            nc.gpsimd.partition_broadcast(bc[:], rec[:])
            OTa = ot_pool.tile([64, 512], f32, tag="ota")
            nc.vector.tensor_mul(OTa[:], CTs[0:64, 0:512], bc[0:64, 0:512])
            OTb = ot_pool.tile([64, 512], f32, tag="otb")
            nc.vector.tensor_mul(OTb[:], CTs[0:64, 512:1024],
                                 bc[0:64, 512:1024])
            nc.sync.dma_start(
                out=outT[(2 * g) * 64:(2 * g + 1) * 64, s0:s0 + SCH],
                in_=OTa[:])
            nc.sync.dma_start(
                out=outT[(2 * g + 1) * 64:(2 * g + 2) * 64, s0:s0 + SCH],
                in_=OTb[:])

        CT = [None]

        def emit_pv(s, ex, exoff):
            g, c, j = steps[s]
            _, _, v3 = pair[g]
            if j == 0:
                ct_t = ps_ct.tile([128, 1024], f32, tag="ct", name="ct_t")
                CT[0] = ct_t
            nc.tensor.matmul(
                CT[0][0:VW, 0:512],
                lhsT=v3[:, j, 0:VW],
                rhs=ex[:, exoff:exoff + 512],
                start=(j == 0), stop=(j == NT - 1))
            nc.tensor.matmul(
                CT[0][0:VW, 512:1024],
                lhsT=v3[:, j, VW:2 * VW],
                rhs=ex[:, exoff + 512:exoff + 1024],
                start=(j == 0), stop=(j == NT - 1))

        def emit_tail(s):
            # per-step chunk bookkeeping: deferred finalize + chunk-end
            # evacuation of CT (DVE-only part)
            g, c, j = steps[s]
            if j == 8 and pend:
                emit_finalize2()
            if j != NT - 1:
                return
            if fin_mode == "none":
                dcons = fin_pool.tile([1, 8], f32, tag="dcons")
                nc.vector.tensor_copy(dcons[:], CT[0][0:1, 0:8])
                nc.sync.dma_start(
                    out=outT[200:201,
                             (g * NCH + c) * 8:(g * NCH + c + 1) * 8],
                    in_=dcons[:])
                return
            CTs = fin_pool.tile([65, 1024], f32, tag="cts")
            nc.vector.tensor_copy(CTs[:], CT[0][0:65, :])
            rec = fin_pool.tile([1, 1024], f32, tag="rec")
            nc.vector.reciprocal(rec[:], CTs[64:65, :])
            if fin_mode == "part1":
                dcons = fin_pool.tile([1, 8], f32, tag="dcons")
                nc.vector.tensor_copy(dcons[:], rec[0:1, 0:8])
                nc.sync.dma_start(
                    out=outT[200:201,
                             (g * NCH + c) * 8:(g * NCH + c + 1) * 8],
                    in_=dcons[:])
                return
            pend.append((g, c, CTs, rec))

        sc_cur = emit_qk(0)
        for s in range(len(steps)):
            sc_next = emit_qk(s + 1) if s + 1 < len(steps) else None
            ex = ex_pool.tile([128, 1024], pvd, tag="ex")
            nc.scalar.activation(ex[:], sc_cur[:], EXP)
            emit_pv(s, ex, 0)
            emit_tail(s)
            sc_cur = sc_next

        while pend:
            emit_finalize2()
    nc.compile()
    return nc


def get_nc(reps=1, qk_dt="float32r", pv_dt="float32r", fin_mode="full",
           dma_hoist=False):
    key = ("nc", reps, qk_dt, pv_dt, fin_mode, dma_hoist)
    if key not in _CACHE:
        _CACHE[key] = _build_program(reps, qk_dt, pv_dt, fin_mode, dma_hoist)
    return _CACHE[key]


def _pack_section(arr, dt_name):
    """arr: fp32 [..., n]. Returns a float32-typed view of the section data
    in its on-device dtype (f32r: same width; bf16: half width)."""
    if dt_name == "float32r":
        return _round_fp32r(arr)
    import ml_dtypes
    bf = np.ascontiguousarray(arr, np.float32).astype(ml_dtypes.bfloat16)
    out = np.zeros(arr.shape[:-1] + (arr.shape[-1] // 2,), np.float32)
    out.view(np.uint16)[...] = bf.view(np.uint16)
    return out


def make_in_maps(query, key, value, qk_dt="float32r", pv_dt="float32r"):
    """Host-side sharding + layout prep. Returns list of per-core input maps."""
    query = np.asarray(query, dtype=np.float32)
    key = np.asarray(key, dtype=np.float32)
    value = np.asarray(value, dtype=np.float32)
    in_maps = []
    for c in range(N_CORES):
        b = c // 4
        n0 = HEADS_PER_CORE * (c % 4)
        q = query[:, b, n0:n0 + 4, :]   # [2048, 4, 64]
        k = key[:, b, n0:n0 + 4, :]
        v = value[:, b, n0:n0 + 4, :]
        # [4, 64, 2048] -> per pair rows
        qt = _pack_section(q.transpose(1, 2, 0).reshape(2, 128, SQ), qk_dt)
        kt = _pack_section(k.transpose(1, 2, 0).reshape(2, 128, SQ), qk_dt)
        # V' [2048, 4, 66] -> [16, 128, 2 pairs, 132] -> [2, 128, 16*132]
        vp = np.concatenate(
            [v, np.ones((SQ, 4, 1), np.float32),
             np.zeros((SQ, 4, 1), np.float32)], axis=2)
        vp = vp.reshape(16, 128, 2, 2 * VW).transpose(2, 1, 0, 3)
        vp = _pack_section(vp.reshape(2, 128, 16 * 2 * VW), pv_dt)
        slab = np.concatenate([qt, kt, vp], axis=2)
        in_maps.append({"ins": np.ascontiguousarray(slab)})
    return in_maps


def assemble_output(results):
    """results: list of per-core {name: array} dicts -> full [2048, 2, 1024]."""
    out = np.empty((SQ, B, NHEADS, HN), np.float32)
    for c in range(N_CORES):
        b = c // 4
        n0 = HEADS_PER_CORE * (c % 4)
        oT = np.asarray(results[c]["outT"])  # [256, 2048]
        out[:, b, n0:n0 + 4, :] = oT.reshape(4, HN, SQ).transpose(2, 0, 1)
    return out.reshape(SQ, B, NHEADS * HN)


def kernel(query, key, value):
    try:
        from concourse.bass_utils import run_bass_kernel_spmd
    except ImportError:
        import sys
        sys.path.insert(0, "/opt/trn_rl_repo")
        from concourse.bass_utils import run_bass_kernel_spmd

    nc = get_nc()
    in_maps = make_in_maps(query, key, value)
    res = run_bass_kernel_spmd(nc, in_maps, list(range(N_CORES)))
    return assemble_output(res.results)
